# revision 34
# baseline (speedup 1.0000x reference)
"""Trainium2 Bass kernel for nn_AttentionBlock (B=4, C=256, H=W=64, 4 heads,
GroupNorm(16) + qkv 1x1 + attention + proj 1x1 + residual).

Sharding: 16 (batch, head) units across 8 cores -> 2 heads (same batch) per
core. Each core computes GroupNorm + qkv for its batch (replicated across the
2 cores sharing a batch), attention for its 2 heads, and a partial proj over
its 128 input channels. Host sums the two partials per batch.

Design (measured 379-382us vs the 428us v1 baseline):
 - softmax exp split 32/32 across ACT (true exp) and DVE (Schraudolph
   int16-bitcast exp, ~3% band that cancels through the softmax): the
   285us single-engine ACT wall drops to ~40us/step per engine. DVE tiles
   are emitted as two 512-col halves so the qk psum frees ~600ns earlier
   (the 2-deep psum ring leaves only ~300ns of slack).
 - AV as bf16 [65, 512] matmuls: 64 v rows + a ones row that accumulates
   the softmax denominator for free (fp8/DoubleRow paths measure no faster:
   DoubleRow streams 1 col/cycle on HW and the M<=64 limit forces separate
   denominator matmuls, which cost exactly the saved cycles).
 - single fused S3 loop: QK+exp(tci) with the AV chains of the same tci
   lagged 2 j-groups behind; leftover AV work carries into the next step so
   the PE never drains at a step boundary. proj(tci-1) runs at groups 12-15
   (its stages' normalize chain takes ~10us of cross-engine latency).
 - normalize: st-copy (ACT) -> den row to partition 0 (gpsimd DMA) ->
   reciprocal_approx (DVE) -> batched partition broadcasts (Pool - its ONLY
   op type, since every gpsimd LOAD_LIB switch blocks that queue ~6us) ->
   batched muls (DVE). proj bias is folded into the residual tensor on the
   host, so the residual is a single DVE add from psum.
"""
import os
import numpy as np
import ml_dtypes
from contextlib import ExitStack

import concourse.bass as bass
import concourse.bacc as bacc
import concourse.tile as tile
from concourse import mybir
from concourse.bass_utils import run_bass_kernel_spmd

F32 = mybir.dt.float32
F32R = mybir.dt.float32r
BF16 = mybir.dt.bfloat16
I16 = mybir.dt.int16

B, C, HH, WW = 4, 256, 64, 64
T = HH * WW          # 4096
EPS = 1e-5
N_CORES = 8
TC = 1024            # attention t-chunk (exp tile width)
NTC = T // TC        # 4 t-chunks
LN2 = float(np.log(2.0))
# Schraudolph exp -> bf16 bits: bits = s * 0.125 * 128/ln2 + (16256 - sigma)
SCH_A = 0.125 * 128.0 / LN2      # 23.0831
SCH_B = 16256.0 - 5.6


def _emit(tc_ctx):
    nc = tc_ctx.nc
    tc = tc_ctx

    d_xb = nc.dram_tensor("xb", [2, 128, T], F32, kind="ExternalInput").ap()
    d_wqk = nc.dram_tensor("wqk", [2, 128, 256], BF16, kind="ExternalInput").ap()
    d_bqk = nc.dram_tensor("bqk", [128, 2], F32, kind="ExternalInput").ap()
    d_wv = nc.dram_tensor("wv", [2, 128, 128], BF16, kind="ExternalInput").ap()
    d_bv = nc.dram_tensor("bv", [1, 128], BF16, kind="ExternalInput").ap()
    d_gnw = nc.dram_tensor("gnw", [128, 2], F32, kind="ExternalInput").ap()
    d_gnb = nc.dram_tensor("gnb", [128, 2], F32, kind="ExternalInput").ap()
    d_gagg = nc.dram_tensor("gagg", [128, 128], F32R, kind="ExternalInput").ap()
    d_pw = nc.dram_tensor("pw", [2, 64, 256], F32R, kind="ExternalInput").ap()
    d_conesb = nc.dram_tensor("conesb", [128, 128], BF16, kind="ExternalInput").ap()
    d_out = nc.dram_tensor("out", [2, 128, T], F32, kind="ExternalOutput").ap()

    with ExitStack() as ctx:
        persist = ctx.enter_context(tc.tile_pool(name="persist", bufs=1))
        small = ctx.enter_context(tc.tile_pool(name="small", bufs=1))

        # ---- persistent SBUF ----
        xb_sb = [persist.tile([128, T], F32, tag=f"xb{i}", name=f"xb_sb{i}")
                 for i in range(2)]
        q_sb = persist.tile([128, T], BF16, tag="qsb")
        k_sb = persist.tile([128, T], BF16, tag="ksb")
        # vtb[p, j, h, c]: c<64: v[ch c, s = j*128 + p]; c=64: ones row
        # (the ones row accumulates the softmax denominator for free)
        vtb = persist.tile([128, 32, 2, 65], BF16, tag="vtb")

        w_wqk = small.tile([128, 2, 256], BF16, tag="wqk")
        w_wv = small.tile([128, 2, 128], BF16, tag="wv")
        onesb = small.tile([1, 128], BF16, tag="onesb")
        w_gagg = small.tile([128, 128], F32R, tag="gagg")
        w_pw = [small.tile([64, 256], F32R, tag=f"pw{i}", name=f"w_pw{i}")
                for i in range(2)]
        b_qk = small.tile([128, 2], F32, tag="bqk")
        b_v = small.tile([1, 128], BF16, tag="bv")
        b_gnw = small.tile([128, 2], F32, tag="gnw")
        b_gnb = small.tile([128, 2], F32, tag="gnb")
        t_eps = small.tile([128, 1], F32, tag="eps")

        normedb = []

        # ================= S1: load x + GroupNorm =================
        with tc.tile_pool(name="gn_ps", bufs=2, space="PSUM") as gn_ps, \
             tc.tile_pool(name="gn_tmp", bufs=4) as gn_tmp:
            for ct in range(2):
                for sub in range(8):
                    eng = (nc.sync, nc.gpsimd, nc.scalar, nc.gpsimd)[sub % 4]
                    eng.dma_start(xb_sb[ct][:, sub * 512:(sub + 1) * 512],
                                  d_xb[ct, :, sub * 512:(sub + 1) * 512])
            nc.sync.dma_start(w_wqk[:], d_wqk.rearrange("k c o -> c k o"))
            nc.sync.dma_start(w_wv[:], d_wv.rearrange("k c o -> c k o"))
            nc.sync.dma_start(w_gagg[:], d_gagg)
            nc.sync.dma_start(onesb[:], d_conesb[0:1, :])
            for i in range(2):
                nc.sync.dma_start(w_pw[i][:], d_pw[i])
            nc.sync.dma_start(b_qk[:], d_bqk)
            nc.sync.dma_start(b_v[:], d_bv)
            nc.sync.dma_start(b_gnw[:], d_gnw)
            nc.sync.dma_start(b_gnb[:], d_gnb)
            nc.vector.memset(t_eps[:], EPS / 4)
            nc.vector.memset(
                vtb[:, :, :, 64:65].rearrange("p a b c -> p (a b) c"), 1.0)
            # preload the gpsimd broadcast library (LOAD_LIB costs ~6us and
            # would otherwise block the first normalize chain)
            warm_src = gn_tmp.tile([1, 8], F32, tag="warms")
            nc.vector.memset(warm_src[:], 0.0)
            warm = gn_tmp.tile([64, 8], F32, tag="warm")
            nc.gpsimd.partition_broadcast(warm[:], warm_src[:], channels=64)
            cbs = [persist.tile([128, T], BF16, tag=f"nb{i}",
                                name=f"normedb{i}") for i in range(2)]
            for ct in range(2):
                xt = xb_sb[ct]
                sin = gn_tmp.tile([128, 2], F32R, tag="sin")
                stats = gn_tmp.tile([128, 8, 6], F32, tag="stats")
                xv = xt[:].rearrange("p (n f) -> p n f", f=512)
                for sub in range(8):
                    nc.vector.bn_stats(stats[:, sub, :], xv[:, sub, :])
                mv = gn_tmp.tile([128, 2], F32, tag="mv")
                nc.vector.bn_aggr(mv[:], stats[:])
                msq = gn_tmp.tile([128, 1], F32, tag="msq")
                nc.vector.tensor_mul(msq[:], mv[:, 0:1], mv[:, 0:1])
                nc.vector.tensor_copy(sin[:, 0:1], mv[:, 0:1])
                nc.vector.tensor_add(sin[:, 1:2], mv[:, 1:2], msq[:])
                ps_g = gn_ps.tile([128, 2], F32, tag="gps")
                nc.tensor.matmul(ps_g[:], w_gagg[:], sin[:], start=True, stop=True)
                g_sb = gn_tmp.tile([128, 2], F32, tag="gsb")
                nc.vector.tensor_copy(g_sb[:], ps_g[:])
                gm2 = gn_tmp.tile([128, 1], F32, tag="gm2")
                nc.vector.tensor_mul(gm2[:], g_sb[:, 0:1], g_sb[:, 0:1])
                gvar = gn_tmp.tile([128, 1], F32, tag="gvar")
                nc.vector.tensor_sub(gvar[:], g_sb[:, 1:2], gm2[:])
                srt = gn_tmp.tile([128, 1], F32, tag="srt")
                nc.scalar.activation(srt[:], gvar[:],
                                     mybir.ActivationFunctionType.Sqrt,
                                     bias=t_eps[:], scale=1.0)
                rstd = gn_tmp.tile([128, 1], F32, tag="rstd")
                nc.vector.reciprocal(rstd[:], srt[:])
                # fold (x-gm)*rstd*w + b into one ACT pass: x*sc + bi
                sc = gn_tmp.tile([128, 1], F32, tag="sc")
                nc.vector.tensor_mul(sc[:], rstd[:], b_gnw[:, ct:ct + 1])
                bi = gn_tmp.tile([128, 1], F32, tag="bi")
                nc.vector.tensor_mul(bi[:], g_sb[:, 0:1], sc[:])
                nc.vector.tensor_sub(bi[:], b_gnb[:, ct:ct + 1], bi[:])
                cb = cbs[ct]
                if ct == 0:
                    nc.scalar.activation(cb[:], xt[:],
                                         mybir.ActivationFunctionType.Identity,
                                         bias=bi[:], scale=sc[:])
                else:
                    # second half on DVE so both GN normalizes run in parallel
                    nc.vector.tensor_scalar(
                        out=cb[:], in0=xt[:], scalar1=sc[:], scalar2=bi[:],
                        op0=mybir.AluOpType.mult, op1=mybir.AluOpType.add)
                normedb.append(cb)

        # ================= S2: qkv + v transpose =================
        qk_dst = [q_sb, k_sb]
        with tc.tile_pool(name="qkv_ps", bufs=3, space="PSUM") as qkv_ps, \
             tc.tile_pool(name="vt_ps", bufs=3, space="PSUM") as vt_ps:
            for ot in range(2):
                for chk in range(8):
                    ps = qkv_ps.tile([128, 512], F32, tag="qkv")
                    for kt in range(2):
                        nc.tensor.matmul(
                            ps[:], w_wqk[:, kt, ot * 128:(ot + 1) * 128],
                            normedb[kt][:, chk * 512:(chk + 1) * 512],
                            start=(kt == 0), stop=(kt == 1))
                    dst = qk_dst[ot][:, chk * 512:(chk + 1) * 512]
                    if chk % 2 == 0:
                        nc.scalar.activation(
                            dst, ps[:], mybir.ActivationFunctionType.Identity,
                            bias=b_qk[:, ot:ot + 1], scale=1.0)
                    else:
                        nc.vector.tensor_scalar(
                            out=dst, in0=ps[:],
                            scalar1=b_qk[:, ot:ot + 1], scalar2=None,
                            op0=mybir.AluOpType.add)
            # vT direct: out[s, hc] = normed[:, s].T @ Wv[:, hc] + bv (ones row)
            for chk in range(T // 128):
                pvt = vt_ps.tile([128, 128], F32, tag="vt")
                for kt in range(2):
                    nc.tensor.matmul(
                        pvt[:], normedb[kt][:, chk * 128:(chk + 1) * 128],
                        w_wv[:, kt, :], start=(kt == 0), stop=False)
                nc.tensor.matmul(pvt[:], onesb[:], b_v[:],
                                 start=False, stop=True)
                dst = vtb[:, chk, :, 0:64]
                src = pvt[:].rearrange("p (h c) -> p h c", h=2)
                if chk % 2 == 0:
                    nc.scalar.copy(dst, src)
                else:
                    nc.vector.tensor_copy(dst, src)

        # ================= S3: fused attention pipeline =================
        mm_pool = ctx.enter_context(tc.tile_pool(name="mm", bufs=2, space="PSUM"))
        av_pool = ctx.enter_context(tc.tile_pool(name="av", bufs=4, space="PSUM"))
        exp_pool = ctx.enter_context(tc.tile_pool(name="exp", bufs=32))
        st_pool = ctx.enter_context(tc.tile_pool(name="st", bufs=8))
        rsb_pool = ctx.enter_context(tc.tile_pool(name="rsb", bufs=4))
        bcr_pool = ctx.enter_context(tc.tile_pool(name="bcr", bufs=4))
        osb_pool = ctx.enter_context(tc.tile_pool(name="osb", bufs=2))

        exp_tiles = {}
        stages = {}
        rsbs = {}

        def emit_qk(tci, j, h, engine):
            """QK psum + exp for s-tile j (128 rows), head h."""
            qs = mm_pool.tile([128, TC], F32, tag="mm", name=f"qk{tci}_{j}_{h}")
            for n2 in range(TC // 512):
                nc.tensor.matmul(
                    qs[:, n2 * 512:(n2 + 1) * 512],
                    k_sb[h * 64:(h + 1) * 64, j * 128:(j + 1) * 128],
                    q_sb[h * 64:(h + 1) * 64,
                         tci * TC + n2 * 512: tci * TC + (n2 + 1) * 512],
                    start=True, stop=True)
            et = exp_pool.tile([128, TC], BF16, tag="exp", name=f"e{tci}_{j}_{h}")
            exp_tiles[(tci, j, h)] = et
            if engine == "act":
                for n2 in range(2):
                    nc.scalar.activation(et[:, n2 * 512:(n2 + 1) * 512],
                                         qs[:, n2 * 512:(n2 + 1) * 512],
                                         mybir.ActivationFunctionType.Exp,
                                         scale=0.125)
            else:
                # two 512-halves: releases the qk psum ~600ns earlier than a
                # single 1024-wide op (the psum-reuse margin is ~300ns)
                for n2 in range(2):
                    nc.vector.tensor_scalar(
                        out=et[:, n2 * 512:(n2 + 1) * 512].bitcast(I16),
                        in0=qs[:, n2 * 512:(n2 + 1) * 512],
                        scalar1=SCH_A, scalar2=SCH_B,
                        op0=mybir.AluOpType.mult, op1=mybir.AluOpType.add)

        def av_chain_work(tci):
            """(j_tag, closure) list: 4 chains (h, half), each 32 bf16 matmuls
            [65, 512] accumulating over j, then st-copy + recip + broadcast +
            normalize. Item j runs once exp(tci, j, *) exists (lag 2 groups)."""
            chains = []
            for h in range(2):
                for half in range(2):
                    avt = av_pool.tile([65, 512], F32, tag="av",
                                       name=f"av{tci}_{h}_{half}")
                    chains.append((h, half, avt))
            work = []
            for j in range(32):
                for h, half, avt in chains:
                    def mm(j=j, h=h, half=half, avt=avt, tci=tci):
                        rhs = exp_tiles[(tci, j, h)][
                            :, half * 512:(half + 1) * 512]
                        nc.tensor.matmul(
                            avt[:], vtb[:, j, h, :], rhs,
                            start=(j == 0), stop=(j == 31),
                            skip_group_check=True)
                    work.append((j, mm))
                    if j == 31:
                        work.append((32, _mk_fin(tci, h, half, avt)))
            return work

        def _mk_fin(tci, h, half, avt):
            # stage A of the normalize chain: st-copy + den row to p0 + recip
            def fin():
                st = st_pool.tile([65, 512], F32R, tag="st",
                                  name=f"st{tci}_{h}_{half}")
                stages[(tci, h, half)] = st
                nc.scalar.copy(st[:], avt[:])
                dn0 = rsb_pool.tile([1, 512], F32, tag="dn0",
                                    name=f"d{tci}_{h}_{half}")
                nc.gpsimd.dma_start(dn0[:], st[64:65, :].bitcast(F32))
                rsb = rsb_pool.tile([1, 512], F32, tag="rsb",
                                    name=f"r{tci}_{h}_{half}")
                nc.vector.reciprocal_approx_fast(rsb[:], dn0[:])
                rsbs[(tci, h, half)] = rsb
            return fin

        def _mk_finb(tci):
            # stage B, batched: 4 broadcasts on Pool (its only lib -> no
            # 6us LOAD_LIB thrash), then 4 normalize muls on DVE
            def finb():
                bcrs = []
                for h in range(2):
                    for half in range(2):
                        bcr = bcr_pool.tile([64, 512], F32, tag="bcr",
                                            name=f"b{tci}_{h}_{half}")
                        nc.gpsimd.partition_broadcast(
                            bcr[:], rsbs[(tci, h, half)][:], channels=64)
                        bcrs.append((h, half, bcr))
                for h, half, bcr in bcrs:
                    st = stages[(tci, h, half)]
                    nc.vector.tensor_mul(st[0:64, :], st[0:64, :], bcr[:])
            return finb

        def proj_work(tci):
            """Closure list: proj + bias + residual + store for tci."""
            work = []
            pstore = {}
            for ot in range(2):
                def mms(ot=ot, tci=tci):
                    ps = mm_pool.tile([128, TC], F32, tag="mm",
                                      name=f"pj{tci}_{ot}")
                    pstore[ot] = ps
                    for half in range(2):
                        for h in range(2):
                            nc.tensor.matmul(
                                ps[:, half * 512:(half + 1) * 512],
                                w_pw[h][:, ot * 128:(ot + 1) * 128],
                                stages[(tci, h, half)][0:64, :],
                                start=(h == 0), stop=(h == 1))
                work.append(mms)

                def fin(ot=ot, tci=tci):
                    ps = pstore[ot]
                    osb = osb_pool.tile([128, TC], F32, tag="osb",
                                        name=f"o{tci}_{ot}")
                    nc.vector.tensor_add(osb[:], ps[:],
                                         xb_sb[ot][:, tci * TC:(tci + 1) * TC])
                    (nc.sync if ot == 0 else nc.scalar).dma_start(
                        d_out[ot, :, tci * TC:(tci + 1) * TC], osb[:])
                work.append(fin)
            return work

        av_q = []           # carried (j, closure) items across steps
        finb_q = []         # deferred batched-normalize closures

        def pop_av(limit, jmax):
            n = 0
            while n < limit and av_q:
                tag_tci, j, w = av_q[0]
                if tag_tci == cur_step and j > jmax:
                    break
                av_q.pop(0)
                w()
                n += 1

        for step in range(NTC + 1):
            cur_step = step
            if step < NTC:
                av_q.extend((step, j, w) for j, w in av_chain_work(step))
                finb_q.append(_mk_finb(step))
            pj_list = proj_work(step - 1) if step >= 1 else []
            pji = 0
            if step < NTC:
                for g in range(16):
                    if g == 0:
                        # drain carried work (incl. the normalize fins) FIRST
                        # so the fin chain enters the engine queues ahead of
                        # this step's exp work
                        pop_av(24, -1)
                    # ACT on the outer claims, DVE inner: best measured
                    # (strict alternation and 3-ACT groups both regress)
                    engs = ("act", "dve", "dve", "act")
                    emit_qk(step, 2 * g, 0, engs[0])
                    emit_qk(step, 2 * g, 1, engs[1])
                    pop_av(4, 2 * g - 4)
                    emit_qk(step, 2 * g + 1, 0, engs[2])
                    emit_qk(step, 2 * g + 1, 1, engs[3])
                    pop_av(4, 2 * g - 3)
                    if g == 8 and len(finb_q) > 1:
                        finb_q.pop(0)()
                    if g >= 12 and pji < len(pj_list):
                        pj_list[pji](); pji += 1
            # drain: finish remaining av (all of it on the last step), then
            # interleave with remaining proj
            if step == NTC:
                pop_av(10 ** 9, 10 ** 9)
                while finb_q:
                    finb_q.pop(0)()
            while (av_q and av_q[0][0] < step) or pji < len(pj_list):
                pop_av(8, -1)
                if pji < len(pj_list):
                    pj_list[pji](); pji += 1


_NC_CACHE = None


def build_nc():
    global _NC_CACHE
    if _NC_CACHE is not None:
        return _NC_CACHE
    nc = bacc.Bacc("TRN2", target_bir_lowering=False, debug=False,
                   num_devices=N_CORES)
    with tile.TileContext(nc) as t:
        _emit(t)
    nc.compile()
    _NC_CACHE = nc
    return nc


def make_core_inputs(inputs, core):
    x = np.ascontiguousarray(np.asarray(inputs["x"], np.float32))
    norm_w = np.asarray(inputs["norm_w"], np.float32)
    norm_b = np.asarray(inputs["norm_b"], np.float32)
    qkv_w = np.asarray(inputs["qkv_w"], np.float32)
    qkv_b = np.asarray(inputs["qkv_b"], np.float32)
    proj_w = np.asarray(inputs["proj_w"], np.float32)
    proj_b = np.asarray(inputs["proj_b"], np.float32)
    b, p = core // 2, core % 2
    ha, hb = 2 * p, 2 * p + 1
    x2 = x.reshape(B, C, T)

    def rows(h, part):
        base = 192 * h + 64 * part
        return slice(base, base + 64)

    xb = np.ascontiguousarray(
        (0.5 * x2[b] + 0.5 * proj_b[:, None]).reshape(2, 128, T))
    # o-tile 0 = [q_ha, q_hb], o-tile 1 = [k_ha, k_hb]
    wqk_rows = np.concatenate([qkv_w[rows(ha, 0)], qkv_w[rows(hb, 0)],
                               qkv_w[rows(ha, 1)], qkv_w[rows(hb, 1)]], axis=0)
    wqk = np.ascontiguousarray(wqk_rows.T.reshape(2, 128, 256)).astype(ml_dtypes.bfloat16)
    bqk = np.ascontiguousarray(
        np.concatenate([qkv_b[rows(ha, 0)], qkv_b[rows(hb, 0)],
                        qkv_b[rows(ha, 1)], qkv_b[rows(hb, 1)]]).reshape(2, 128).T)
    wv_rows = np.concatenate([qkv_w[rows(ha, 2)], qkv_w[rows(hb, 2)]], axis=0)
    wv = np.ascontiguousarray(wv_rows.T.reshape(2, 128, 128)).astype(ml_dtypes.bfloat16)
    bv = np.ascontiguousarray(
        np.concatenate([qkv_b[rows(ha, 2)],
                        qkv_b[rows(hb, 2)]]).reshape(1, 128)).astype(ml_dtypes.bfloat16)
    gnw = np.ascontiguousarray(norm_w.reshape(2, 128).T)
    gnb = np.ascontiguousarray(norm_b.reshape(2, 128).T)
    gagg = np.kron(np.eye(8, dtype=np.float32),
                   np.ones((16, 16), np.float32) / 16.0)
    pw = np.ascontiguousarray(
        proj_w[:, 128 * p:128 * p + 128].T.reshape(2, 64, 256))
    conesb = np.ones((128, 128), ml_dtypes.bfloat16)
    return dict(xb=xb, wqk=wqk, bqk=bqk, wv=wv, bv=bv, gnw=gnw, gnb=gnb,
                gagg=gagg, pw=pw, conesb=conesb)


def _ensure_axon_devices():
    """The SPMD run needs the 8 axon-tunneled NeuronCores visible to jax.
    If a caller pinned jax to cpu (e.g. to run the reference), try to undo."""
    import jax
    try:
        if len(jax.devices("axon")) >= N_CORES:
            return
    except Exception:
        pass
    try:
        os.environ.pop("JAX_PLATFORMS", None)
        jax.config.update("jax_platforms", None)
        jax.extend.backend.clear_backends()
    except Exception:
        pass


def kernel(**inputs):
    try:
        import jax
        if not any(d.platform == "axon" for d in jax.devices()):
            _ensure_axon_devices()
    except Exception:
        _ensure_axon_devices()
    nc = build_nc()
    in_maps = [make_core_inputs(inputs, core) for core in range(N_CORES)]
    res = None
    last_err = None
    for attempt in range(4):
        try:
            res = run_bass_kernel_spmd(nc, in_maps, list(range(N_CORES)))
            break
        except Exception as e:  # transient NRT_EXEC_UNIT_UNRECOVERABLE etc.
            last_err = e
            import time as _time
            _time.sleep(2.0)
    if res is None:
        raise last_err
    out = np.empty((B, C, T), np.float32)
    for b in range(B):
        out[b] = (res.results[2 * b]["out"].reshape(C, T)
                  + res.results[2 * b + 1]["out"].reshape(C, T))
    return out.reshape(B, C, HH, WW)


# revision 36
# speedup vs baseline: 1.1410x; 1.1410x over previous
"""Trainium2 Bass kernel for nn_AttentionBlock (B=4, C=256, H=W=64, 4 heads,
GroupNorm(16) + qkv 1x1 + attention + proj 1x1 + residual).

Sharding: 16 (batch, head) units across 8 cores -> 2 heads (same batch) per
core. Each core computes GroupNorm + qkv for its batch (replicated across the
2 cores sharing a batch), attention for its 2 heads, and a partial proj over
its 128 input channels. Host sums the two partials per batch.

Design (measured 379-382us vs the 428us v1 baseline):
 - softmax exp split 32/32 across ACT (true exp) and DVE (Schraudolph
   int16-bitcast exp, ~3% band that cancels through the softmax): the
   285us single-engine ACT wall drops to ~40us/step per engine. DVE tiles
   are emitted as two 512-col halves so the qk psum frees ~600ns earlier
   (the 2-deep psum ring leaves only ~300ns of slack).
 - AV as bf16 [65, 512] matmuls: 64 v rows + a ones row that accumulates
   the softmax denominator for free (fp8/DoubleRow paths measure no faster:
   DoubleRow streams 1 col/cycle on HW and the M<=64 limit forces separate
   denominator matmuls, which cost exactly the saved cycles).
 - single fused S3 loop: QK+exp(tci) with the AV chains of the same tci
   lagged 2 j-groups behind; leftover AV work carries into the next step so
   the PE never drains at a step boundary. proj(tci-1) runs at groups 12-15
   (its stages' normalize chain takes ~10us of cross-engine latency).
 - normalize: st-copy (ACT) -> den row to partition 0 (gpsimd DMA) ->
   reciprocal_approx (DVE) -> batched partition broadcasts (Pool - its ONLY
   op type, since every gpsimd LOAD_LIB switch blocks that queue ~6us) ->
   batched muls (DVE). proj bias is folded into the residual tensor on the
   host, so the residual is a single DVE add from psum.
"""
import os
import numpy as np
import ml_dtypes
from contextlib import ExitStack

import concourse.bass as bass
import concourse.bacc as bacc
import concourse.tile as tile
from concourse import mybir
from concourse.bass_utils import run_bass_kernel_spmd

F32 = mybir.dt.float32
F32R = mybir.dt.float32r
BF16 = mybir.dt.bfloat16
I16 = mybir.dt.int16

B, C, HH, WW = 4, 256, 64, 64
T = HH * WW          # 4096
EPS = 1e-5
N_CORES = 8
TC = 1024            # attention t-chunk (exp tile width)
NTC = T // TC        # 4 t-chunks
LN2 = float(np.log(2.0))
# Schraudolph exp -> bf16 bits: bits = s * 0.125 * 128/ln2 + (16256 - sigma)
SCH_A = 0.125 * 128.0 / LN2      # 23.0831
SCH_B = 16256.0 - 5.6


def _emit(tc_ctx):
    nc = tc_ctx.nc
    tc = tc_ctx

    d_xb = nc.dram_tensor("xb", [2, 128, T], F32, kind="ExternalInput").ap()
    d_wqk = nc.dram_tensor("wqk", [2, 128, 256], BF16, kind="ExternalInput").ap()
    d_bqk = nc.dram_tensor("bqk", [128, 2], F32, kind="ExternalInput").ap()
    d_wv = nc.dram_tensor("wv", [2, 128, 128], BF16, kind="ExternalInput").ap()
    d_bv = nc.dram_tensor("bv", [1, 128], BF16, kind="ExternalInput").ap()
    d_gnw = nc.dram_tensor("gnw", [128, 2], F32, kind="ExternalInput").ap()
    d_gnb = nc.dram_tensor("gnb", [128, 2], F32, kind="ExternalInput").ap()
    d_gagg = nc.dram_tensor("gagg", [128, 128], F32R, kind="ExternalInput").ap()
    d_pw = nc.dram_tensor("pw", [2, 64, 256], F32R, kind="ExternalInput").ap()
    d_conesb = nc.dram_tensor("conesb", [128, 128], BF16, kind="ExternalInput").ap()
    d_out = nc.dram_tensor("out", [2, 128, T], F32, kind="ExternalOutput").ap()

    with ExitStack() as ctx:
        persist = ctx.enter_context(tc.tile_pool(name="persist", bufs=1))
        small = ctx.enter_context(tc.tile_pool(name="small", bufs=1))

        # ---- persistent SBUF ----
        xb_sb = [persist.tile([128, T], F32, tag=f"xb{i}", name=f"xb_sb{i}")
                 for i in range(2)]
        q_sb = persist.tile([128, T], BF16, tag="qsb")
        k_sb = persist.tile([128, T], BF16, tag="ksb")
        # vtb[p, j, h, c]: c<64: v[ch c, s = j*128 + p]; c=64: ones row
        # (the ones row accumulates the softmax denominator for free)
        vtb = persist.tile([128, 32, 2, 65], BF16, tag="vtb")

        w_wqk = small.tile([128, 2, 256], BF16, tag="wqk")
        w_wv = small.tile([128, 2, 128], BF16, tag="wv")
        onesb = small.tile([1, 128], BF16, tag="onesb")
        w_gagg = small.tile([128, 128], F32R, tag="gagg")
        w_pw = [small.tile([64, 256], F32R, tag=f"pw{i}", name=f"w_pw{i}")
                for i in range(2)]
        b_qk = small.tile([128, 2], F32, tag="bqk")
        b_v = small.tile([1, 128], BF16, tag="bv")
        b_gnw = small.tile([128, 2], F32, tag="gnw")
        b_gnb = small.tile([128, 2], F32, tag="gnb")
        t_eps = small.tile([128, 1], F32, tag="eps")

        normedb = []

        # ================= S1: load x + GroupNorm =================
        with tc.tile_pool(name="gn_ps", bufs=2, space="PSUM") as gn_ps, \
             tc.tile_pool(name="gn_tmp", bufs=4) as gn_tmp:
            for ct in range(2):
                for sub in range(8):
                    eng = (nc.sync, nc.gpsimd, nc.scalar, nc.gpsimd)[sub % 4]
                    eng.dma_start(xb_sb[ct][:, sub * 512:(sub + 1) * 512],
                                  d_xb[ct, :, sub * 512:(sub + 1) * 512])
            nc.sync.dma_start(w_wqk[:], d_wqk.rearrange("k c o -> c k o"))
            nc.sync.dma_start(w_wv[:], d_wv.rearrange("k c o -> c k o"))
            nc.sync.dma_start(w_gagg[:], d_gagg)
            nc.sync.dma_start(onesb[:], d_conesb[0:1, :])
            for i in range(2):
                nc.sync.dma_start(w_pw[i][:], d_pw[i])
            nc.sync.dma_start(b_qk[:], d_bqk)
            nc.sync.dma_start(b_v[:], d_bv)
            nc.sync.dma_start(b_gnw[:], d_gnw)
            nc.sync.dma_start(b_gnb[:], d_gnb)
            nc.vector.memset(t_eps[:], EPS / 4)
            nc.vector.memset(
                vtb[:, :, :, 64:65].rearrange("p a b c -> p (a b) c"), 1.0)
            # preload the gpsimd broadcast library (LOAD_LIB costs ~6us and
            # would otherwise block the first normalize chain)
            warm_src = gn_tmp.tile([1, 8], F32, tag="warms")
            nc.vector.memset(warm_src[:], 0.0)
            warm = gn_tmp.tile([64, 8], F32, tag="warm")
            nc.gpsimd.partition_broadcast(warm[:], warm_src[:], channels=64)
            cbs = [persist.tile([128, T], BF16, tag=f"nb{i}",
                                name=f"normedb{i}") for i in range(2)]
            for ct in range(2):
                xt = xb_sb[ct]
                sin = gn_tmp.tile([128, 2], F32R, tag="sin")
                stats = gn_tmp.tile([128, 8, 6], F32, tag="stats")
                xv = xt[:].rearrange("p (n f) -> p n f", f=512)
                for sub in range(8):
                    nc.vector.bn_stats(stats[:, sub, :], xv[:, sub, :])
                mv = gn_tmp.tile([128, 2], F32, tag="mv")
                nc.vector.bn_aggr(mv[:], stats[:])
                msq = gn_tmp.tile([128, 1], F32, tag="msq")
                nc.vector.tensor_mul(msq[:], mv[:, 0:1], mv[:, 0:1])
                nc.vector.tensor_copy(sin[:, 0:1], mv[:, 0:1])
                nc.vector.tensor_add(sin[:, 1:2], mv[:, 1:2], msq[:])
                ps_g = gn_ps.tile([128, 2], F32, tag="gps")
                nc.tensor.matmul(ps_g[:], w_gagg[:], sin[:], start=True, stop=True)
                g_sb = gn_tmp.tile([128, 2], F32, tag="gsb")
                nc.vector.tensor_copy(g_sb[:], ps_g[:])
                gm2 = gn_tmp.tile([128, 1], F32, tag="gm2")
                nc.vector.tensor_mul(gm2[:], g_sb[:, 0:1], g_sb[:, 0:1])
                gvar = gn_tmp.tile([128, 1], F32, tag="gvar")
                nc.vector.tensor_sub(gvar[:], g_sb[:, 1:2], gm2[:])
                srt = gn_tmp.tile([128, 1], F32, tag="srt")
                nc.scalar.activation(srt[:], gvar[:],
                                     mybir.ActivationFunctionType.Sqrt,
                                     bias=t_eps[:], scale=1.0)
                rstd = gn_tmp.tile([128, 1], F32, tag="rstd")
                nc.vector.reciprocal(rstd[:], srt[:])
                # fold (x-gm)*rstd*w + b into one ACT pass: x*sc + bi
                sc = gn_tmp.tile([128, 1], F32, tag="sc")
                nc.vector.tensor_mul(sc[:], rstd[:], b_gnw[:, ct:ct + 1])
                bi = gn_tmp.tile([128, 1], F32, tag="bi")
                nc.vector.tensor_mul(bi[:], g_sb[:, 0:1], sc[:])
                nc.vector.tensor_sub(bi[:], b_gnb[:, ct:ct + 1], bi[:])
                cb = cbs[ct]
                if ct == 0:
                    nc.scalar.activation(cb[:], xt[:],
                                         mybir.ActivationFunctionType.Identity,
                                         bias=bi[:], scale=sc[:])
                else:
                    # second half on DVE so both GN normalizes run in parallel
                    nc.vector.tensor_scalar(
                        out=cb[:], in0=xt[:], scalar1=sc[:], scalar2=bi[:],
                        op0=mybir.AluOpType.mult, op1=mybir.AluOpType.add)
                normedb.append(cb)

        # ================= S2: qkv + v transpose =================
        qk_dst = [q_sb, k_sb]
        with tc.tile_pool(name="qkv_ps", bufs=3, space="PSUM") as qkv_ps, \
             tc.tile_pool(name="vt_ps", bufs=3, space="PSUM") as vt_ps:
            for ot in range(2):
                for chk in range(8):
                    ps = qkv_ps.tile([128, 512], F32, tag="qkv")
                    for kt in range(2):
                        nc.tensor.matmul(
                            ps[:], w_wqk[:, kt, ot * 128:(ot + 1) * 128],
                            normedb[kt][:, chk * 512:(chk + 1) * 512],
                            start=(kt == 0), stop=(kt == 1))
                    dst = qk_dst[ot][:, chk * 512:(chk + 1) * 512]
                    if chk % 2 == 0:
                        nc.scalar.activation(
                            dst, ps[:], mybir.ActivationFunctionType.Identity,
                            bias=b_qk[:, ot:ot + 1], scale=1.0)
                    else:
                        nc.vector.tensor_scalar(
                            out=dst, in0=ps[:],
                            scalar1=b_qk[:, ot:ot + 1], scalar2=None,
                            op0=mybir.AluOpType.add)
            # vT direct: out[s, hc] = normed[:, s].T @ Wv[:, hc] + bv (ones row)
            for chk in range(T // 128):
                pvt = vt_ps.tile([128, 128], F32, tag="vt")
                for kt in range(2):
                    nc.tensor.matmul(
                        pvt[:], normedb[kt][:, chk * 128:(chk + 1) * 128],
                        w_wv[:, kt, :], start=(kt == 0), stop=False)
                nc.tensor.matmul(pvt[:], onesb[:], b_v[:],
                                 start=False, stop=True)
                dst = vtb[:, chk, :, 0:64]
                src = pvt[:].rearrange("p (h c) -> p h c", h=2)
                if chk % 2 == 0:
                    nc.scalar.copy(dst, src)
                else:
                    nc.vector.tensor_copy(dst, src)

        # ================= S3: fused attention pipeline =================
        mm_pool = ctx.enter_context(tc.tile_pool(name="mm", bufs=2, space="PSUM"))
        av_pool = ctx.enter_context(tc.tile_pool(name="av", bufs=4, space="PSUM"))
        exp_pool = ctx.enter_context(tc.tile_pool(name="exp", bufs=32))
        st_pool = ctx.enter_context(tc.tile_pool(name="st", bufs=8))
        rsb_pool = ctx.enter_context(tc.tile_pool(name="rsb", bufs=4))
        bcr_pool = ctx.enter_context(tc.tile_pool(name="bcr", bufs=4))
        osb_pool = ctx.enter_context(tc.tile_pool(name="osb", bufs=2))

        exp_tiles = {}
        stages = {}
        rsbs = {}

        def emit_qk(tci, j, h, engine):
            """QK psum + exp for s-tile j (128 rows), head h."""
            qs = mm_pool.tile([128, TC], F32, tag="mm", name=f"qk{tci}_{j}_{h}")
            for n2 in range(TC // 512):
                nc.tensor.matmul(
                    qs[:, n2 * 512:(n2 + 1) * 512],
                    k_sb[h * 64:(h + 1) * 64, j * 128:(j + 1) * 128],
                    q_sb[h * 64:(h + 1) * 64,
                         tci * TC + n2 * 512: tci * TC + (n2 + 1) * 512],
                    start=True, stop=True)
            et = exp_pool.tile([128, TC], BF16, tag="exp", name=f"e{tci}_{j}_{h}")
            exp_tiles[(tci, j, h)] = et
            if engine == "act":
                nc.scalar.activation(et[:], qs[:],
                                     mybir.ActivationFunctionType.Exp,
                                     scale=0.125)
            else:
                # two 512-halves: releases the qk psum ~600ns earlier than a
                # single 1024-wide op (the psum-reuse margin is ~300ns)
                for n2 in range(2):
                    nc.vector.tensor_scalar(
                        out=et[:, n2 * 512:(n2 + 1) * 512].bitcast(I16),
                        in0=qs[:, n2 * 512:(n2 + 1) * 512],
                        scalar1=SCH_A, scalar2=SCH_B,
                        op0=mybir.AluOpType.mult, op1=mybir.AluOpType.add)

        def av_chain_work(tci):
            """(j_tag, closure) list: 4 chains (h, half), each 32 bf16 matmuls
            [65, 512] accumulating over j, then st-copy + recip + broadcast +
            normalize. Item j runs once exp(tci, j, *) exists (lag 2 groups)."""
            chains = []
            for h in range(2):
                for half in range(2):
                    avt = av_pool.tile([65, 512], F32, tag="av",
                                       name=f"av{tci}_{h}_{half}")
                    chains.append((h, half, avt))
            work = []
            for j in range(32):
                for h, half, avt in chains:
                    def mm(j=j, h=h, half=half, avt=avt, tci=tci):
                        rhs = exp_tiles[(tci, j, h)][
                            :, half * 512:(half + 1) * 512]
                        nc.tensor.matmul(
                            avt[:], vtb[:, j, h, :], rhs,
                            start=(j == 0), stop=(j == 31),
                            skip_group_check=True)
                    work.append((j, mm))
                    if j == 31:
                        work.append((32, _mk_fin(tci, h, half, avt)))
            return work

        def _mk_fin(tci, h, half, avt):
            # stage A of the normalize chain: st-copy + den row to p0 + recip
            def fin():
                st = st_pool.tile([65, 512], F32R, tag="st",
                                  name=f"st{tci}_{h}_{half}")
                stages[(tci, h, half)] = st
                if half == 0:
                    nc.scalar.copy(st[:], avt[:])
                else:
                    nc.vector.tensor_copy(st[:], avt[:])
                dn0 = rsb_pool.tile([1, 512], F32, tag="dn0",
                                    name=f"d{tci}_{h}_{half}")
                nc.gpsimd.dma_start(dn0[:], st[64:65, :].bitcast(F32))
                rsb = rsb_pool.tile([1, 512], F32, tag="rsb",
                                    name=f"r{tci}_{h}_{half}")
                nc.vector.reciprocal_approx_fast(rsb[:], dn0[:])
                rsbs[(tci, h, half)] = rsb
            return fin

        def _mk_finb(tci):
            # stage B, batched: 4 broadcasts on Pool (its only lib -> no
            # 6us LOAD_LIB thrash), then 4 normalize muls on DVE
            def finb():
                bcrs = []
                for h in range(2):
                    for half in range(2):
                        bcr = bcr_pool.tile([64, 512], F32, tag="bcr",
                                            name=f"b{tci}_{h}_{half}")
                        nc.gpsimd.partition_broadcast(
                            bcr[:], rsbs[(tci, h, half)][:], channels=64)
                        bcrs.append((h, half, bcr))
                for h, half, bcr in bcrs:
                    st = stages[(tci, h, half)]
                    nc.vector.tensor_mul(st[0:64, :], st[0:64, :], bcr[:])
            return finb

        def proj_work(tci):
            """Closure list: proj + bias + residual + store for tci."""
            work = []
            pstore = {}
            for ot in range(2):
                def mms(ot=ot, tci=tci):
                    ps = mm_pool.tile([128, TC], F32, tag="mm",
                                      name=f"pj{tci}_{ot}")
                    pstore[ot] = ps
                    for half in range(2):
                        for h in range(2):
                            nc.tensor.matmul(
                                ps[:, half * 512:(half + 1) * 512],
                                w_pw[h][:, ot * 128:(ot + 1) * 128],
                                stages[(tci, h, half)][0:64, :],
                                start=(h == 0), stop=(h == 1))
                work.append(mms)

                def fin(ot=ot, tci=tci):
                    ps = pstore[ot]
                    osb = osb_pool.tile([128, TC], F32, tag="osb",
                                        name=f"o{tci}_{ot}")
                    nc.vector.tensor_add(osb[:], ps[:],
                                         xb_sb[ot][:, tci * TC:(tci + 1) * TC])
                    (nc.sync if ot == 0 else nc.scalar).dma_start(
                        d_out[ot, :, tci * TC:(tci + 1) * TC], osb[:])
                work.append(fin)
            return work

        av_q = []           # carried (j, closure) items across steps
        finb_q = []         # deferred batched-normalize closures

        def pop_av(limit, jmax):
            n = 0
            while n < limit and av_q:
                tag_tci, j, w = av_q[0]
                if tag_tci == cur_step and j > jmax:
                    break
                av_q.pop(0)
                w()
                n += 1

        for step in range(NTC + 1):
            cur_step = step
            if step < NTC:
                av_q.extend((step, j, w) for j, w in av_chain_work(step))
                finb_q.append(_mk_finb(step))
            pj_list = proj_work(step - 1) if step >= 1 else []
            pji = 0
            if step < NTC:
                for g in range(16):
                    if g == 0:
                        # drain carried work (incl. the normalize fins) FIRST
                        # so the fin chain enters the engine queues ahead of
                        # this step's exp work
                        pop_av(24, -1)
                    # ACT on the outer claims, DVE inner: best measured
                    # (strict alternation and 3-ACT groups both regress)
                    engs = ("act", "dve", "dve", "act")
                    emit_qk(step, 2 * g, 0, engs[0])
                    emit_qk(step, 2 * g, 1, engs[1])
                    pop_av(4, 2 * g - 4)
                    emit_qk(step, 2 * g + 1, 0, engs[2])
                    emit_qk(step, 2 * g + 1, 1, engs[3])
                    pop_av(4, 2 * g - 3)
                    if g == 8 and len(finb_q) > 1:
                        finb_q.pop(0)()
                    if g >= 12 and pji < len(pj_list):
                        pj_list[pji](); pji += 1
            # drain: finish remaining av (all of it on the last step), then
            # interleave with remaining proj
            if step == NTC:
                pop_av(10 ** 9, 10 ** 9)
                while finb_q:
                    finb_q.pop(0)()
            while (av_q and av_q[0][0] < step) or pji < len(pj_list):
                pop_av(8, -1)
                if pji < len(pj_list):
                    pj_list[pji](); pji += 1


_NC_CACHE = None


def build_nc():
    global _NC_CACHE
    if _NC_CACHE is not None:
        return _NC_CACHE
    nc = bacc.Bacc("TRN2", target_bir_lowering=False, debug=False,
                   num_devices=N_CORES)
    with tile.TileContext(nc) as t:
        _emit(t)
    nc.compile()
    _NC_CACHE = nc
    return nc


def make_core_inputs(inputs, core):
    x = np.ascontiguousarray(np.asarray(inputs["x"], np.float32))
    norm_w = np.asarray(inputs["norm_w"], np.float32)
    norm_b = np.asarray(inputs["norm_b"], np.float32)
    qkv_w = np.asarray(inputs["qkv_w"], np.float32)
    qkv_b = np.asarray(inputs["qkv_b"], np.float32)
    proj_w = np.asarray(inputs["proj_w"], np.float32)
    proj_b = np.asarray(inputs["proj_b"], np.float32)
    b, p = core // 2, core % 2
    ha, hb = 2 * p, 2 * p + 1
    x2 = x.reshape(B, C, T)

    def rows(h, part):
        base = 192 * h + 64 * part
        return slice(base, base + 64)

    xb = np.ascontiguousarray(
        (0.5 * x2[b] + 0.5 * proj_b[:, None]).reshape(2, 128, T))
    # o-tile 0 = [q_ha, q_hb], o-tile 1 = [k_ha, k_hb]
    wqk_rows = np.concatenate([qkv_w[rows(ha, 0)], qkv_w[rows(hb, 0)],
                               qkv_w[rows(ha, 1)], qkv_w[rows(hb, 1)]], axis=0)
    wqk = np.ascontiguousarray(wqk_rows.T.reshape(2, 128, 256)).astype(ml_dtypes.bfloat16)
    bqk = np.ascontiguousarray(
        np.concatenate([qkv_b[rows(ha, 0)], qkv_b[rows(hb, 0)],
                        qkv_b[rows(ha, 1)], qkv_b[rows(hb, 1)]]).reshape(2, 128).T)
    wv_rows = np.concatenate([qkv_w[rows(ha, 2)], qkv_w[rows(hb, 2)]], axis=0)
    wv = np.ascontiguousarray(wv_rows.T.reshape(2, 128, 128)).astype(ml_dtypes.bfloat16)
    bv = np.ascontiguousarray(
        np.concatenate([qkv_b[rows(ha, 2)],
                        qkv_b[rows(hb, 2)]]).reshape(1, 128)).astype(ml_dtypes.bfloat16)
    gnw = np.ascontiguousarray(norm_w.reshape(2, 128).T)
    gnb = np.ascontiguousarray(norm_b.reshape(2, 128).T)
    gagg = np.kron(np.eye(8, dtype=np.float32),
                   np.ones((16, 16), np.float32) / 16.0)
    pw = np.ascontiguousarray(
        proj_w[:, 128 * p:128 * p + 128].T.reshape(2, 64, 256))
    conesb = np.ones((128, 128), ml_dtypes.bfloat16)
    return dict(xb=xb, wqk=wqk, bqk=bqk, wv=wv, bv=bv, gnw=gnw, gnb=gnb,
                gagg=gagg, pw=pw, conesb=conesb)


def _ensure_axon_devices():
    """The SPMD run needs the 8 axon-tunneled NeuronCores visible to jax.
    If a caller pinned jax to cpu (e.g. to run the reference), try to undo."""
    import jax
    try:
        if len(jax.devices("axon")) >= N_CORES:
            return
    except Exception:
        pass
    try:
        os.environ.pop("JAX_PLATFORMS", None)
        jax.config.update("jax_platforms", None)
        jax.extend.backend.clear_backends()
    except Exception:
        pass


def kernel(**inputs):
    try:
        import jax
        if not any(d.platform == "axon" for d in jax.devices()):
            _ensure_axon_devices()
    except Exception:
        _ensure_axon_devices()
    nc = build_nc()
    in_maps = [make_core_inputs(inputs, core) for core in range(N_CORES)]
    res = None
    last_err = None
    for attempt in range(4):
        try:
            res = run_bass_kernel_spmd(nc, in_maps, list(range(N_CORES)))
            break
        except Exception as e:  # transient NRT_EXEC_UNIT_UNRECOVERABLE etc.
            last_err = e
            import time as _time
            _time.sleep(2.0)
    if res is None:
        raise last_err
    out = np.empty((B, C, T), np.float32)
    for b in range(B):
        out[b] = (res.results[2 * b]["out"].reshape(C, T)
                  + res.results[2 * b + 1]["out"].reshape(C, T))
    return out.reshape(B, C, HH, WW)


# revision 37
# speedup vs baseline: 1.1629x; 1.0192x over previous
"""Trainium2 Bass kernel for nn_AttentionBlock (B=4, C=256, H=W=64, 4 heads,
GroupNorm(16) + qkv 1x1 + attention + proj 1x1 + residual).

Sharding: 16 (batch, head) units across 8 cores -> 2 heads (same batch) per
core. Each core computes GroupNorm + qkv for its batch (replicated across the
2 cores sharing a batch), attention for its 2 heads, and a partial proj over
its 128 input channels. Host sums the two partials per batch.

Design (measured 379-382us vs the 428us v1 baseline):
 - softmax exp split 32/32 across ACT (true exp) and DVE (Schraudolph
   int16-bitcast exp, ~3% band that cancels through the softmax): the
   285us single-engine ACT wall drops to ~40us/step per engine. DVE tiles
   are emitted as two 512-col halves so the qk psum frees ~600ns earlier
   (the 2-deep psum ring leaves only ~300ns of slack).
 - AV as bf16 [65, 512] matmuls: 64 v rows + a ones row that accumulates
   the softmax denominator for free (fp8/DoubleRow paths measure no faster:
   DoubleRow streams 1 col/cycle on HW and the M<=64 limit forces separate
   denominator matmuls, which cost exactly the saved cycles).
 - single fused S3 loop: QK+exp(tci) with the AV chains of the same tci
   lagged 2 j-groups behind; leftover AV work carries into the next step so
   the PE never drains at a step boundary. proj(tci-1) runs at groups 12-15
   (its stages' normalize chain takes ~10us of cross-engine latency).
 - normalize: st-copy (ACT) -> den row to partition 0 (gpsimd DMA) ->
   reciprocal_approx (DVE) -> batched partition broadcasts (Pool - its ONLY
   op type, since every gpsimd LOAD_LIB switch blocks that queue ~6us) ->
   batched muls (DVE). proj bias is folded into the residual tensor on the
   host, so the residual is a single DVE add from psum.
"""
import os
import numpy as np
import ml_dtypes
from contextlib import ExitStack

import concourse.bass as bass
import concourse.bacc as bacc
import concourse.tile as tile
from concourse import mybir
from concourse.bass_utils import run_bass_kernel_spmd

F32 = mybir.dt.float32
F32R = mybir.dt.float32r
BF16 = mybir.dt.bfloat16
I16 = mybir.dt.int16

B, C, HH, WW = 4, 256, 64, 64
T = HH * WW          # 4096
EPS = 1e-5
N_CORES = 8
TC = 1024            # attention t-chunk (exp tile width)
NTC = T // TC        # 4 t-chunks
LN2 = float(np.log(2.0))
# Schraudolph exp -> bf16 bits: bits = s * 0.125 * 128/ln2 + (16256 - sigma)
SCH_A = 0.125 * 128.0 / LN2      # 23.0831
SCH_B = 16256.0 - 5.6


def _emit(tc_ctx):
    nc = tc_ctx.nc
    tc = tc_ctx

    d_xb = nc.dram_tensor("xb", [2, 128, T], F32, kind="ExternalInput").ap()
    d_wqk = nc.dram_tensor("wqk", [2, 128, 256], BF16, kind="ExternalInput").ap()
    d_bqk = nc.dram_tensor("bqk", [128, 2], F32, kind="ExternalInput").ap()
    d_wv = nc.dram_tensor("wv", [2, 128, 128], BF16, kind="ExternalInput").ap()
    d_bv = nc.dram_tensor("bv", [1, 128], BF16, kind="ExternalInput").ap()
    d_gnw = nc.dram_tensor("gnw", [128, 2], F32, kind="ExternalInput").ap()
    d_gnb = nc.dram_tensor("gnb", [128, 2], F32, kind="ExternalInput").ap()
    d_gagg = nc.dram_tensor("gagg", [128, 128], F32R, kind="ExternalInput").ap()
    d_pw = nc.dram_tensor("pw", [2, 64, 256], F32R, kind="ExternalInput").ap()
    d_conesb = nc.dram_tensor("conesb", [128, 128], BF16, kind="ExternalInput").ap()
    d_out = nc.dram_tensor("out", [2, 128, T], F32, kind="ExternalOutput").ap()

    with ExitStack() as ctx:
        persist = ctx.enter_context(tc.tile_pool(name="persist", bufs=1))
        small = ctx.enter_context(tc.tile_pool(name="small", bufs=1))

        # ---- persistent SBUF ----
        xb_sb = [persist.tile([128, T], F32, tag=f"xb{i}", name=f"xb_sb{i}")
                 for i in range(2)]
        q_sb = persist.tile([128, T], BF16, tag="qsb")
        k_sb = persist.tile([128, T], BF16, tag="ksb")
        # vtb[p, j, h, c]: c<64: v[ch c, s = j*128 + p]; c=64: ones row
        # (the ones row accumulates the softmax denominator for free)
        vtb = persist.tile([128, 32, 2, 65], BF16, tag="vtb")

        w_wqk = small.tile([128, 2, 256], BF16, tag="wqk")
        w_wv = small.tile([128, 2, 128], BF16, tag="wv")
        onesb = small.tile([1, 128], BF16, tag="onesb")
        w_gagg = small.tile([128, 128], F32R, tag="gagg")
        w_pw = [small.tile([64, 256], F32R, tag=f"pw{i}", name=f"w_pw{i}")
                for i in range(2)]
        b_qk = small.tile([128, 2], F32, tag="bqk")
        b_v = small.tile([1, 128], BF16, tag="bv")
        b_gnw = small.tile([128, 2], F32, tag="gnw")
        b_gnb = small.tile([128, 2], F32, tag="gnb")
        t_eps = small.tile([128, 1], F32, tag="eps")

        normedb = []

        # ================= S1: load x + GroupNorm =================
        with tc.tile_pool(name="gn_ps", bufs=2, space="PSUM") as gn_ps, \
             tc.tile_pool(name="gn_tmp", bufs=4) as gn_tmp:
            for ct in range(2):
                for sub in range(8):
                    eng = (nc.sync, nc.gpsimd, nc.scalar, nc.gpsimd)[sub % 4]
                    eng.dma_start(xb_sb[ct][:, sub * 512:(sub + 1) * 512],
                                  d_xb[ct, :, sub * 512:(sub + 1) * 512])
            nc.sync.dma_start(w_wqk[:], d_wqk.rearrange("k c o -> c k o"))
            nc.sync.dma_start(w_wv[:], d_wv.rearrange("k c o -> c k o"))
            nc.sync.dma_start(w_gagg[:], d_gagg)
            nc.sync.dma_start(onesb[:], d_conesb[0:1, :])
            for i in range(2):
                nc.sync.dma_start(w_pw[i][:], d_pw[i])
            nc.sync.dma_start(b_qk[:], d_bqk)
            nc.sync.dma_start(b_v[:], d_bv)
            nc.sync.dma_start(b_gnw[:], d_gnw)
            nc.sync.dma_start(b_gnb[:], d_gnb)
            nc.vector.memset(t_eps[:], EPS / 4)
            nc.vector.memset(
                vtb[:, :, :, 64:65].rearrange("p a b c -> p (a b) c"), 1.0)
            # preload the gpsimd broadcast library (LOAD_LIB costs ~6us and
            # would otherwise block the first normalize chain)
            warm_src = gn_tmp.tile([1, 8], F32, tag="warms")
            nc.vector.memset(warm_src[:], 0.0)
            warm = gn_tmp.tile([64, 8], F32, tag="warm")
            nc.gpsimd.partition_broadcast(warm[:], warm_src[:], channels=64)
            cbs = [persist.tile([128, T], BF16, tag=f"nb{i}",
                                name=f"normedb{i}") for i in range(2)]
            for ct in range(2):
                xt = xb_sb[ct]
                sin = gn_tmp.tile([128, 2], F32R, tag="sin")
                stats = gn_tmp.tile([128, 8, 6], F32, tag="stats")
                xv = xt[:].rearrange("p (n f) -> p n f", f=512)
                for sub in range(8):
                    nc.vector.bn_stats(stats[:, sub, :], xv[:, sub, :])
                mv = gn_tmp.tile([128, 2], F32, tag="mv")
                nc.vector.bn_aggr(mv[:], stats[:])
                msq = gn_tmp.tile([128, 1], F32, tag="msq")
                nc.vector.tensor_mul(msq[:], mv[:, 0:1], mv[:, 0:1])
                nc.vector.tensor_copy(sin[:, 0:1], mv[:, 0:1])
                nc.vector.tensor_add(sin[:, 1:2], mv[:, 1:2], msq[:])
                ps_g = gn_ps.tile([128, 2], F32, tag="gps")
                nc.tensor.matmul(ps_g[:], w_gagg[:], sin[:], start=True, stop=True)
                g_sb = gn_tmp.tile([128, 2], F32, tag="gsb")
                nc.vector.tensor_copy(g_sb[:], ps_g[:])
                gm2 = gn_tmp.tile([128, 1], F32, tag="gm2")
                nc.vector.tensor_mul(gm2[:], g_sb[:, 0:1], g_sb[:, 0:1])
                gvar = gn_tmp.tile([128, 1], F32, tag="gvar")
                nc.vector.tensor_sub(gvar[:], g_sb[:, 1:2], gm2[:])
                srt = gn_tmp.tile([128, 1], F32, tag="srt")
                nc.scalar.activation(srt[:], gvar[:],
                                     mybir.ActivationFunctionType.Sqrt,
                                     bias=t_eps[:], scale=1.0)
                rstd = gn_tmp.tile([128, 1], F32, tag="rstd")
                nc.vector.reciprocal(rstd[:], srt[:])
                # fold (x-gm)*rstd*w + b into one ACT pass: x*sc + bi
                sc = gn_tmp.tile([128, 1], F32, tag="sc")
                nc.vector.tensor_mul(sc[:], rstd[:], b_gnw[:, ct:ct + 1])
                bi = gn_tmp.tile([128, 1], F32, tag="bi")
                nc.vector.tensor_mul(bi[:], g_sb[:, 0:1], sc[:])
                nc.vector.tensor_sub(bi[:], b_gnb[:, ct:ct + 1], bi[:])
                cb = cbs[ct]
                # chunked normalize (8x512) so S2's qkv matmuls can start on
                # chunk 0 while later chunks are still being normalized;
                # engines alternate by (ct, chunk) to run both cts in parallel
                for chk in range(8):
                    sl = slice(chk * 512, (chk + 1) * 512)
                    if (chk + ct) % 2 == 0:
                        nc.scalar.activation(cb[:, sl], xt[:, sl],
                                             mybir.ActivationFunctionType.Identity,
                                             bias=bi[:], scale=sc[:])
                    else:
                        nc.vector.tensor_scalar(
                            out=cb[:, sl], in0=xt[:, sl], scalar1=sc[:],
                            scalar2=bi[:], op0=mybir.AluOpType.mult,
                            op1=mybir.AluOpType.add)
                normedb.append(cb)

        # ================= S2: qkv + v transpose =================
        qk_dst = [q_sb, k_sb]
        with tc.tile_pool(name="qkv_ps", bufs=3, space="PSUM") as qkv_ps, \
             tc.tile_pool(name="vt_ps", bufs=3, space="PSUM") as vt_ps:
            for ot in range(2):
                for chk in range(8):
                    ps = qkv_ps.tile([128, 512], F32, tag="qkv")
                    for kt in range(2):
                        nc.tensor.matmul(
                            ps[:], w_wqk[:, kt, ot * 128:(ot + 1) * 128],
                            normedb[kt][:, chk * 512:(chk + 1) * 512],
                            start=(kt == 0), stop=(kt == 1))
                    dst = qk_dst[ot][:, chk * 512:(chk + 1) * 512]
                    if chk % 2 == 0:
                        nc.scalar.activation(
                            dst, ps[:], mybir.ActivationFunctionType.Identity,
                            bias=b_qk[:, ot:ot + 1], scale=1.0)
                    else:
                        nc.vector.tensor_scalar(
                            out=dst, in0=ps[:],
                            scalar1=b_qk[:, ot:ot + 1], scalar2=None,
                            op0=mybir.AluOpType.add)
            # vT direct: out[s, hc] = normed[:, s].T @ Wv[:, hc] + bv (ones row)
            for chk in range(T // 128):
                pvt = vt_ps.tile([128, 128], F32, tag="vt")
                for kt in range(2):
                    nc.tensor.matmul(
                        pvt[:], normedb[kt][:, chk * 128:(chk + 1) * 128],
                        w_wv[:, kt, :], start=(kt == 0), stop=False)
                nc.tensor.matmul(pvt[:], onesb[:], b_v[:],
                                 start=False, stop=True)
                dst = vtb[:, chk, :, 0:64]
                src = pvt[:].rearrange("p (h c) -> p h c", h=2)
                if chk % 2 == 0:
                    nc.scalar.copy(dst, src)
                else:
                    nc.vector.tensor_copy(dst, src)

        # ================= S3: fused attention pipeline =================
        mm_pool = ctx.enter_context(tc.tile_pool(name="mm", bufs=2, space="PSUM"))
        av_pool = ctx.enter_context(tc.tile_pool(name="av", bufs=4, space="PSUM"))
        exp_pool = ctx.enter_context(tc.tile_pool(name="exp", bufs=32))
        st_pool = ctx.enter_context(tc.tile_pool(name="st", bufs=8))
        rsb_pool = ctx.enter_context(tc.tile_pool(name="rsb", bufs=4))
        bcr_pool = ctx.enter_context(tc.tile_pool(name="bcr", bufs=4))
        osb_pool = ctx.enter_context(tc.tile_pool(name="osb", bufs=2))

        exp_tiles = {}
        stages = {}
        rsbs = {}

        def emit_qk(tci, j, h, engine):
            """QK psum + exp for s-tile j (128 rows), head h."""
            qs = mm_pool.tile([128, TC], F32, tag="mm", name=f"qk{tci}_{j}_{h}")
            for n2 in range(TC // 512):
                nc.tensor.matmul(
                    qs[:, n2 * 512:(n2 + 1) * 512],
                    k_sb[h * 64:(h + 1) * 64, j * 128:(j + 1) * 128],
                    q_sb[h * 64:(h + 1) * 64,
                         tci * TC + n2 * 512: tci * TC + (n2 + 1) * 512],
                    start=True, stop=True)
            et = exp_pool.tile([128, TC], BF16, tag="exp", name=f"e{tci}_{j}_{h}")
            exp_tiles[(tci, j, h)] = et
            if engine == "act":
                nc.scalar.activation(et[:], qs[:],
                                     mybir.ActivationFunctionType.Exp,
                                     scale=0.125)
            else:
                # two 512-halves: releases the qk psum ~600ns earlier than a
                # single 1024-wide op (the psum-reuse margin is ~300ns)
                for n2 in range(2):
                    nc.vector.tensor_scalar(
                        out=et[:, n2 * 512:(n2 + 1) * 512].bitcast(I16),
                        in0=qs[:, n2 * 512:(n2 + 1) * 512],
                        scalar1=SCH_A, scalar2=SCH_B,
                        op0=mybir.AluOpType.mult, op1=mybir.AluOpType.add)

        def av_chain_work(tci):
            """(j_tag, closure) list: 4 chains (h, half), each 32 bf16 matmuls
            [65, 512] accumulating over j, then st-copy + recip + broadcast +
            normalize. Item j runs once exp(tci, j, *) exists (lag 2 groups)."""
            chains = []
            for h in range(2):
                for half in range(2):
                    avt = av_pool.tile([65, 512], F32, tag="av",
                                       name=f"av{tci}_{h}_{half}")
                    chains.append((h, half, avt))
            work = []
            for j in range(32):
                for h, half, avt in chains:
                    def mm(j=j, h=h, half=half, avt=avt, tci=tci):
                        rhs = exp_tiles[(tci, j, h)][
                            :, half * 512:(half + 1) * 512]
                        nc.tensor.matmul(
                            avt[:], vtb[:, j, h, :], rhs,
                            start=(j == 0), stop=(j == 31),
                            skip_group_check=True)
                    work.append((j, mm))
                    if j == 31:
                        work.append((32, _mk_fin(tci, h, half, avt)))
            return work

        def _mk_fin(tci, h, half, avt):
            # stage A of the normalize chain: st-copy + den row to p0 + recip
            def fin():
                st = st_pool.tile([65, 512], F32R, tag="st",
                                  name=f"st{tci}_{h}_{half}")
                stages[(tci, h, half)] = st
                nc.scalar.copy(st[:], avt[:])
                dn0 = rsb_pool.tile([1, 512], F32, tag="dn0",
                                    name=f"d{tci}_{h}_{half}")
                nc.gpsimd.dma_start(dn0[:], st[64:65, :].bitcast(F32))
                rsb = rsb_pool.tile([1, 512], F32, tag="rsb",
                                    name=f"r{tci}_{h}_{half}")
                nc.vector.reciprocal_approx_fast(rsb[:], dn0[:])
                rsbs[(tci, h, half)] = rsb
            return fin

        def _mk_finb(tci):
            # stage B, batched: 4 broadcasts on Pool (its only lib -> no
            # 6us LOAD_LIB thrash), then 4 normalize muls on DVE
            def finb():
                bcrs = []
                for h in range(2):
                    for half in range(2):
                        bcr = bcr_pool.tile([64, 512], F32, tag="bcr",
                                            name=f"b{tci}_{h}_{half}")
                        nc.gpsimd.partition_broadcast(
                            bcr[:], rsbs[(tci, h, half)][:], channels=64)
                        bcrs.append((h, half, bcr))
                for h, half, bcr in bcrs:
                    st = stages[(tci, h, half)]
                    nc.vector.tensor_mul(st[0:64, :], st[0:64, :], bcr[:])
            return finb

        def proj_work(tci):
            """Closure list: proj + bias + residual + store for tci."""
            work = []
            pstore = {}
            for ot in range(2):
                def mms(ot=ot, tci=tci):
                    ps = mm_pool.tile([128, TC], F32, tag="mm",
                                      name=f"pj{tci}_{ot}")
                    pstore[ot] = ps
                    for half in range(2):
                        for h in range(2):
                            nc.tensor.matmul(
                                ps[:, half * 512:(half + 1) * 512],
                                w_pw[h][:, ot * 128:(ot + 1) * 128],
                                stages[(tci, h, half)][0:64, :],
                                start=(h == 0), stop=(h == 1))
                work.append(mms)

                def fin(ot=ot, tci=tci):
                    ps = pstore[ot]
                    osb = osb_pool.tile([128, TC], F32, tag="osb",
                                        name=f"o{tci}_{ot}")
                    nc.vector.tensor_add(osb[:], ps[:],
                                         xb_sb[ot][:, tci * TC:(tci + 1) * TC])
                    (nc.sync if ot == 0 else nc.scalar).dma_start(
                        d_out[ot, :, tci * TC:(tci + 1) * TC], osb[:])
                work.append(fin)
            return work

        av_q = []           # carried (j, closure) items across steps
        finb_q = []         # deferred batched-normalize closures

        def pop_av(limit, jmax):
            n = 0
            while n < limit and av_q:
                tag_tci, j, w = av_q[0]
                if tag_tci == cur_step and j > jmax:
                    break
                av_q.pop(0)
                w()
                n += 1

        for step in range(NTC + 1):
            cur_step = step
            if step < NTC:
                av_q.extend((step, j, w) for j, w in av_chain_work(step))
                finb_q.append(_mk_finb(step))
            pj_list = proj_work(step - 1) if step >= 1 else []
            pji = 0
            if step < NTC:
                for g in range(16):
                    if g == 0:
                        # drain carried work (incl. the normalize fins) FIRST
                        # so the fin chain enters the engine queues ahead of
                        # this step's exp work
                        pop_av(24, -1)
                    # ACT on the outer claims, DVE inner: best measured
                    # (strict alternation and 3-ACT groups both regress)
                    engs = ("act", "dve", "dve", "act")
                    emit_qk(step, 2 * g, 0, engs[0])
                    emit_qk(step, 2 * g, 1, engs[1])
                    pop_av(4, 2 * g - 4)
                    emit_qk(step, 2 * g + 1, 0, engs[2])
                    emit_qk(step, 2 * g + 1, 1, engs[3])
                    pop_av(4, 2 * g - 3)
                    if g == 8 and len(finb_q) > 1:
                        finb_q.pop(0)()
                    if g >= 12 and pji < len(pj_list):
                        pj_list[pji](); pji += 1
            # drain: finish remaining av (all of it on the last step), then
            # interleave with remaining proj
            if step == NTC:
                pop_av(10 ** 9, 10 ** 9)
                while finb_q:
                    finb_q.pop(0)()
            while (av_q and av_q[0][0] < step) or pji < len(pj_list):
                pop_av(8, -1)
                if pji < len(pj_list):
                    pj_list[pji](); pji += 1


_NC_CACHE = None


def build_nc():
    global _NC_CACHE
    if _NC_CACHE is not None:
        return _NC_CACHE
    nc = bacc.Bacc("TRN2", target_bir_lowering=False, debug=False,
                   num_devices=N_CORES)
    with tile.TileContext(nc) as t:
        _emit(t)
    nc.compile()
    _NC_CACHE = nc
    return nc


def make_core_inputs(inputs, core):
    x = np.ascontiguousarray(np.asarray(inputs["x"], np.float32))
    norm_w = np.asarray(inputs["norm_w"], np.float32)
    norm_b = np.asarray(inputs["norm_b"], np.float32)
    qkv_w = np.asarray(inputs["qkv_w"], np.float32)
    qkv_b = np.asarray(inputs["qkv_b"], np.float32)
    proj_w = np.asarray(inputs["proj_w"], np.float32)
    proj_b = np.asarray(inputs["proj_b"], np.float32)
    b, p = core // 2, core % 2
    ha, hb = 2 * p, 2 * p + 1
    x2 = x.reshape(B, C, T)

    def rows(h, part):
        base = 192 * h + 64 * part
        return slice(base, base + 64)

    xb = np.ascontiguousarray(
        (0.5 * x2[b] + 0.5 * proj_b[:, None]).reshape(2, 128, T))
    # o-tile 0 = [q_ha, q_hb], o-tile 1 = [k_ha, k_hb]
    wqk_rows = np.concatenate([qkv_w[rows(ha, 0)], qkv_w[rows(hb, 0)],
                               qkv_w[rows(ha, 1)], qkv_w[rows(hb, 1)]], axis=0)
    wqk = np.ascontiguousarray(wqk_rows.T.reshape(2, 128, 256)).astype(ml_dtypes.bfloat16)
    bqk = np.ascontiguousarray(
        np.concatenate([qkv_b[rows(ha, 0)], qkv_b[rows(hb, 0)],
                        qkv_b[rows(ha, 1)], qkv_b[rows(hb, 1)]]).reshape(2, 128).T)
    wv_rows = np.concatenate([qkv_w[rows(ha, 2)], qkv_w[rows(hb, 2)]], axis=0)
    wv = np.ascontiguousarray(wv_rows.T.reshape(2, 128, 128)).astype(ml_dtypes.bfloat16)
    bv = np.ascontiguousarray(
        np.concatenate([qkv_b[rows(ha, 2)],
                        qkv_b[rows(hb, 2)]]).reshape(1, 128)).astype(ml_dtypes.bfloat16)
    gnw = np.ascontiguousarray(norm_w.reshape(2, 128).T)
    gnb = np.ascontiguousarray(norm_b.reshape(2, 128).T)
    gagg = np.kron(np.eye(8, dtype=np.float32),
                   np.ones((16, 16), np.float32) / 16.0)
    pw = np.ascontiguousarray(
        proj_w[:, 128 * p:128 * p + 128].T.reshape(2, 64, 256))
    conesb = np.ones((128, 128), ml_dtypes.bfloat16)
    return dict(xb=xb, wqk=wqk, bqk=bqk, wv=wv, bv=bv, gnw=gnw, gnb=gnb,
                gagg=gagg, pw=pw, conesb=conesb)


def _ensure_axon_devices():
    """The SPMD run needs the 8 axon-tunneled NeuronCores visible to jax.
    If a caller pinned jax to cpu (e.g. to run the reference), try to undo."""
    import jax
    try:
        if len(jax.devices("axon")) >= N_CORES:
            return
    except Exception:
        pass
    try:
        os.environ.pop("JAX_PLATFORMS", None)
        jax.config.update("jax_platforms", None)
        jax.extend.backend.clear_backends()
    except Exception:
        pass


def kernel(**inputs):
    try:
        import jax
        if not any(d.platform == "axon" for d in jax.devices()):
            _ensure_axon_devices()
    except Exception:
        _ensure_axon_devices()
    nc = build_nc()
    in_maps = [make_core_inputs(inputs, core) for core in range(N_CORES)]
    res = None
    last_err = None
    for attempt in range(4):
        try:
            res = run_bass_kernel_spmd(nc, in_maps, list(range(N_CORES)))
            break
        except Exception as e:  # transient NRT_EXEC_UNIT_UNRECOVERABLE etc.
            last_err = e
            import time as _time
            _time.sleep(2.0)
    if res is None:
        raise last_err
    out = np.empty((B, C, T), np.float32)
    for b in range(B):
        out[b] = (res.results[2 * b]["out"].reshape(C, T)
                  + res.results[2 * b + 1]["out"].reshape(C, T))
    return out.reshape(B, C, HH, WW)
